# revision 5
# baseline (speedup 1.0000x reference)
"""Trainium2 Bass kernel for the sliding-window (sparse block) attention layer.

Problem shape: B=1, C=2048, L=16384, projected c=1024, block bl=512, nb=32
blocks, window 2*bl=1024 with halo bl//2=256.

Sharding: sequence-parallel over the nb block dimension. Each of the 8 cores
owns 4 consecutive blocks (2048 output columns) and receives an overlapping
x1 slab of 2048+2*256 = 2560 columns, so the k/v halo is recomputed locally
and no collectives are needed.

Per-core device pipeline (all matmuls in bf16, f32 PSUM accumulation):
  Phase 1a: k = wk@x1s+bk as (c, 2560) -> DRAM staging (bf16).
  Phase 1b: vT = (x1s^T wv) as (2560, c) directly in transposed layout
            -> DRAM staging (bf16).
  Phase 2 (per block b): qb = wq@x1b+bq (c, 512);
           ST = kb^T qb (keys m on partitions, queries l on free axis);
           P~T = exp(ST/sqrt(c) + logmask) via one ACT op (scale+bias fused);
           denom(l) = sum_m P~T via a DVE add-tree + one gpsimd
           partition_all_reduce (keeps the PE free of the 8 ones-matmuls);
           out = vT^T @ P~T accumulated in PSUM, then x*recip(denom)+bv, relu;
           final = woT^T @ relu(..) + bo -> DMA to DRAM.

Scheduling notes (all pools are flat/top-level so phase boundaries never
release+reallocate SBUF zones):
 - The four weight tensors rotate through ONE 2-slot pool: wk(s0), wv(s1),
   wq(s0), wo(s1). wq's slot frees when the last k-matmul retires (end of
   1a) and its DMAs run during 1b; wo's frees at end of 1b and its DMAs run
   during block 0 of phase 2.
 - Three DMA rings: nc.sync carries the x1 streams + wk/wv (never blocked
   by a waiting descriptor); nc.scalar carries wq/wo and the k/v staging
   reads (these can wait on slot releases without stalling the x1 stream);
   nc.gpsimd (SWDGE) carries all DRAM writes.
 - Kernel start: wk's first 128-column slice is split per-C-chunk and
   interleaved 1:1 with x1 chunk DMAs so the first matmul issues ~1.5us
   after the rings open; the rest of wk follows ci-major so each k-group's
   weights land just ahead of the PE.
"""

import os
import sys

import numpy as np

for _p in ("/root/.axon_site", "/root/.axon_site/_ro/trn_rl_repo", "/opt/trn_rl_repo"):
    if os.path.isdir(_p) and _p not in sys.path:
        sys.path.append(_p)

import ml_dtypes

import concourse.bass as bass
import concourse.bass_isa as bass_isa
import concourse.mybir as mybir
import concourse.tile as tile
from concourse import bacc
from concourse.bass import ds, ts

BF16 = ml_dtypes.bfloat16

# Model dims (hardcoded per problem spec)
C = 2048          # input channels
CQ = 1024         # projected channels
L = 16384         # sequence length
BL = 512          # block length
HALF = 256        # halo = BL // 2
NCORES = 8
LCORE = L // NCORES          # 2048 owned columns per core
LH = LCORE + 2 * HALF        # 2560 slab columns per core
NBLK = LCORE // BL           # 4 blocks per core
WIN = 2 * BL                 # 1024 attention window
NHALF = LH // BL             # 5 window-halves per slab
ESCALE = 1.0 / float(np.sqrt(CQ))  # 1/32

NCI = C // 128    # 16 contraction chunks over C
NCQ = CQ // 128   # 8 chunks over projected c
NCO = C // 128    # 16 chunks over output channels
NMC = WIN // 128  # 8 key chunks per window

# Denominator via gpsimd partition_all_reduce instead of PE ones-matmuls
USE_PAR_DENOM = True


def build_kernel() -> bass.Bass:
    nc = bacc.Bacc("TRN2", target_bir_lowering=False)
    dt = mybir.dt
    f32, bf16 = dt.float32, dt.bfloat16
    AFT = mybir.ActivationFunctionType

    x1s = nc.dram_tensor("x1s", [C, LH], bf16, kind="ExternalInput")
    wqT = nc.dram_tensor("wqT", [C, CQ], bf16, kind="ExternalInput")
    wkT = nc.dram_tensor("wkT", [C, CQ], bf16, kind="ExternalInput")
    wvT = nc.dram_tensor("wvT", [C, CQ], bf16, kind="ExternalInput")
    woT = nc.dram_tensor("woT", [CQ, C], bf16, kind="ExternalInput")
    bq = nc.dram_tensor("bq", [128, CQ // 128], f32, kind="ExternalInput")
    bk = nc.dram_tensor("bk", [128, CQ // 128], f32, kind="ExternalInput")
    bv = nc.dram_tensor("bv", [128, CQ // 128], f32, kind="ExternalInput")
    bo = nc.dram_tensor("bo", [128, C // 128], f32, kind="ExternalInput")
    amask = nc.dram_tensor("amask", [128, NBLK * (WIN // 128)], f32,
                           kind="ExternalInput")
    out = nc.dram_tensor("out", [C, LCORE], f32, kind="ExternalOutput")

    # Internal DRAM staging for k (c-major) and vT (m-major)
    kst = nc.dram_tensor("kst", [128, CQ // 128, LH], bf16)
    vst = nc.dram_tensor("vst", [128, LH // 128, CQ], bf16)

    x1r = x1s.rearrange("(ci p) l -> p ci l", p=128)    # (128, 16, 2560)
    wqr = wqT.rearrange("(ci p) c -> p ci c", p=128)    # (128, 16, 1024)
    wkr = wkT.rearrange("(ci p) c -> p ci c", p=128)
    wvr = wvT.rearrange("(ci p) c -> p ci c", p=128)
    wor = woT.rearrange("(ci p) co -> p ci co", p=128)  # (128, 8, 2048)
    outr = out.rearrange("(co p) l -> p co l", p=128)   # (128, 16, 2048)

    with tile.TileContext(nc) as tc:
        with (
            tc.tile_pool(name="singles", bufs=1) as singles,
            tc.tile_pool(name="wpool", bufs=2) as wpool,
            tc.tile_pool(name="x1pool", bufs=2) as x1pool,
            tc.tile_pool(name="kroll", bufs=2) as krollp,
            tc.tile_pool(name="vroll", bufs=2) as vrollp,
            tc.tile_pool(name="stage", bufs=4) as stage,
            tc.tile_pool(name="qbp", bufs=1) as qbp,
            tc.tile_pool(name="ptp", bufs=8) as ptp,
            tc.tile_pool(name="dl1", bufs=4) as dl1,
            tc.tile_pool(name="dl2", bufs=2) as dl2,
            tc.tile_pool(name="small2", bufs=2) as small2,
            tc.tile_pool(name="relup", bufs=1) as relup,
            tc.tile_pool(name="osbp", bufs=3) as osbp,
            tc.tile_pool(name="tmpp", bufs=2) as tmpp,
            tc.tile_pool(name="psA", bufs=3, space="PSUM") as psA,
            tc.tile_pool(name="psB", bufs=3, space="PSUM") as psB,
            tc.tile_pool(name="psD", bufs=1, space="PSUM") as psD,
        ):
            def load_x1(col0, pre=None):
                """x1 slab tile (128, NCI, BL), split DMAs per C-chunk.
                pre(Ci) lets the caller interleave other DMAs chunk-wise
                (issued after each x1 chunk so x1 wins queue priority)."""
                t = x1pool.tile([128, NCI, BL], bf16, tag="x1")
                for Ci in range(NCI):
                    nc.sync.dma_start(t[:, Ci], x1r[:, Ci, ds(col0, BL)])
                    if pre is not None:
                        pre(Ci)
                return t

            # ---- kernel-start DMAs: wk ci-slice 0 interleaved with x1t0 ----
            wk_sb = wpool.tile([128, NCI, CQ], bf16, tag="w")
            x1t0 = x1pool.tile([128, NCI, BL], bf16, tag="x1")
            for Ci in range(NCI):
                nc.sync.dma_start(wk_sb[:, Ci, ds(0, 128)],
                                  wkr[:, Ci, ds(0, 128)])
                nc.sync.dma_start(x1t0[:, Ci], x1r[:, Ci, ds(0, BL)])

            bq_sb = singles.tile([128, NCQ], f32)
            nc.sync.dma_start(bq_sb, bq[:, :])
            bk_sb = singles.tile([128, NCQ], f32)
            nc.sync.dma_start(bk_sb, bk[:, :])
            bv_sb = singles.tile([128, NCQ], f32)
            nc.sync.dma_start(bv_sb, bv[:, :])
            bo_sb = singles.tile([128, NCO], f32)
            nc.sync.dma_start(bo_sb, bo[:, :])
            am_sb = singles.tile([128, NBLK * NMC], f32)
            nc.sync.dma_start(am_sb, amask[:, :])
            if not USE_PAR_DENOM:
                ones_sb = singles.tile([128, 1], bf16)
                nc.vector.memset(ones_sb, 1.0)

            # rest of wk, ci-major so group ci's weights land just in time
            for ci in range(1, NCQ):
                nc.sync.dma_start(wk_sb[:, :, ts(ci, 128)],
                                  wkr[:, :, ts(ci, 128)])

            wv_sb = wpool.tile([128, NCI, CQ], bf16, tag="w")

            def emit_k(x1t, lc):
                for ci in range(NCQ):
                    ps = psA.tile([128, BL], f32, tag="st")
                    for Ci in range(NCI):
                        nc.tensor.matmul(
                            ps,
                            lhsT=wk_sb[:, Ci, ts(ci, 128)],
                            rhs=x1t[:, Ci, :],
                            start=(Ci == 0),
                            stop=(Ci == NCI - 1),
                        )
                    kt = stage.tile([128, BL], bf16, tag="kst")
                    nc.scalar.add(kt, ps, bk_sb[:, ci:ci + 1])
                    nc.gpsimd.dma_start(kst[:, ci, ts(lc, BL)], kt)

            def emit_v(x1t, lc):
                for mo in range(BL // 128):
                    for ch in range(CQ // BL):
                        ps = psB.tile([128, BL], f32, tag="av")
                        for Ci in range(NCI):
                            nc.tensor.matmul(
                                ps,
                                lhsT=x1t[:, Ci, ts(mo, 128)],
                                rhs=wv_sb[:, Ci, ts(ch, BL)],
                                start=(Ci == 0),
                                stop=(Ci == NCI - 1),
                            )
                        vt = stage.tile([128, BL], bf16, tag="vst")
                        nc.scalar.copy(vt, ps)
                        nc.gpsimd.dma_start(
                            vst[:, lc * 4 + mo, ts(ch, BL)], vt)

            # ---------------- Phase 1a: k -> DRAM ----------------
            for lc in range(NHALF):
                if lc == 0:
                    x1t = x1t0
                else:
                    pre = None
                    if lc == 1:
                        pre = (lambda Ci:
                               nc.sync.dma_start(wv_sb[:, Ci], wvr[:, Ci, :]))
                    x1t = load_x1(lc * BL, pre)
                emit_k(x1t, lc)

            # ---------------- Phase 1b: vT -> DRAM ----------------
            wq_sb = wpool.tile([128, NCI, CQ], bf16, tag="w")
            for lc in range(NHALF):
                pre = None
                if lc == 1:
                    # wq on the scalar ring: waits wk's slot release (end of
                    # 1a) without blocking the x1 stream on the sync ring
                    pre = (lambda Ci:
                           nc.scalar.dma_start(wq_sb[:, Ci], wqr[:, Ci, :]))
                x1t = load_x1(lc * BL, pre)
                emit_v(x1t, lc)

            # ---------------- Phase 2: attention + output proj ----------------
            wo_sb = wpool.tile([128, NCQ, C], bf16, tag="w")

            khalves: dict[int, bass.AP] = {}
            vhalves: dict[int, bass.AP] = {}

            def load_half(h: int):
                kh = krollp.tile([128, NCQ, BL], bf16, tag="kh")
                for ci in range(NCQ):
                    nc.scalar.dma_start(kh[:, ci], kst[:, ci, ts(h, BL)])
                vh = vrollp.tile([128, BL // 128, CQ], bf16, tag="vh")
                for mo in range(BL // 128):
                    nc.scalar.dma_start(vh[:, mo], vst[:, h * 4 + mo, :])
                khalves[h] = kh
                vhalves[h] = vh

            load_half(0)
            load_half(1)
            x1b0 = load_x1(HALF)
            for b in range(NBLK):
                if b == 0:
                    x1b = x1b0
                else:
                    x1b = load_x1(HALF + b * BL)
                    load_half(b + 1)

                # q projection for this block
                qb_sb = qbp.tile([128, NCQ, BL], bf16, tag="qb")
                for ci in range(NCQ):
                    ps = psA.tile([128, BL], f32, tag="st")
                    for Ci in range(NCI):
                        nc.tensor.matmul(
                            ps,
                            lhsT=wq_sb[:, Ci, ts(ci, 128)],
                            rhs=x1b[:, Ci, :],
                            start=(Ci == 0),
                            stop=(Ci == NCI - 1),
                        )
                    nc.scalar.add(qb_sb[:, ci], ps, bq_sb[:, ci:ci + 1])

                # energy^T tiles (keys on partitions) + exp
                pts = []
                lvl1 = []
                if not USE_PAR_DENOM:
                    ps_den = psD.tile([128, BL], f32, tag="den")
                for mc in range(NMC):
                    kh = khalves[b + mc // 4]
                    off = (mc % 4) * 128
                    ps_st = psA.tile([128, BL], f32, tag="st")
                    for ci in range(NCQ):
                        nc.tensor.matmul(
                            ps_st,
                            lhsT=kh[:, ci, ds(off, 128)],
                            rhs=qb_sb[:, ci, :],
                            start=(ci == 0),
                            stop=(ci == NCQ - 1),
                        )
                    pt = ptp.tile([128, BL], bf16, tag="pt")
                    col = b * NMC + mc
                    nc.scalar.activation(
                        pt, ps_st, AFT.Exp,
                        bias=am_sb[:, col:col + 1], scale=ESCALE)
                    pts.append(pt)
                    if USE_PAR_DENOM:
                        # pairwise DVE add tree, built as exps retire
                        if mc % 2 == 1:
                            t = dl1.tile([128, BL], f32, tag="l1")
                            nc.vector.tensor_add(t, pts[mc - 1], pts[mc])
                            lvl1.append(t)
                    else:
                        if mc >= 1:
                            nc.tensor.matmul(
                                ps_den[0:1, :],
                                lhsT=ones_sb,
                                rhs=pts[mc - 1],
                                start=(mc == 1),
                                stop=False,
                            )

                recipb = small2.tile([128, BL], f32, tag="recipb")
                if USE_PAR_DENOM:
                    u0 = dl2.tile([128, BL], f32, tag="l2")
                    nc.vector.tensor_add(u0, lvl1[0], lvl1[1])
                    u1 = dl2.tile([128, BL], f32, tag="l2")
                    nc.vector.tensor_add(u1, lvl1[2], lvl1[3])
                    acc = dl1.tile([128, BL], f32, tag="l1")
                    nc.vector.tensor_add(acc, u0, u1)
                    dsum = small2.tile([128, BL], f32, tag="dsum")
                    nc.gpsimd.partition_all_reduce(
                        dsum, acc, channels=128,
                        reduce_op=bass_isa.ReduceOp.add)
                    nc.vector.reciprocal_approx_fast(recipb, dsum)
                else:
                    nc.tensor.matmul(
                        ps_den[0:1, :], lhsT=ones_sb, rhs=pts[NMC - 1],
                        start=False, stop=True)
                    recip = small2.tile([1, BL], f32, tag="recip")
                    nc.vector.reciprocal_approx_fast(recip, ps_den[0:1, :])
                    nc.gpsimd.partition_broadcast(recipb, recip)

                if b == 0:
                    for ci in range(NCQ):
                        nc.scalar.dma_start(wo_sb[:, ci], wor[:, ci, :])

                # attention * V, divide by denom, +bv, relu
                relu_b = relup.tile([128, NCQ, BL], bf16, tag="relu")
                for ci in range(NCQ):
                    ps_av = psB.tile([128, BL], f32, tag="av")
                    for mc in range(NMC):
                        vh = vhalves[b + mc // 4]
                        nc.tensor.matmul(
                            ps_av,
                            lhsT=vh[:, mc % 4, ts(ci, 128)],
                            rhs=pts[mc],
                            start=(mc == 0),
                            stop=(mc == NMC - 1),
                        )
                    tmp = tmpp.tile([128, BL], f32, tag="tmp")
                    nc.vector.tensor_mul(tmp, ps_av, recipb)
                    nc.scalar.activation(
                        relu_b[:, ci], tmp, AFT.Relu,
                        bias=bv_sb[:, ci:ci + 1], scale=1.0)

                # output projection
                for co in range(NCO):
                    ps_o = psA.tile([128, BL], f32, tag="st")
                    for ci in range(NCQ):
                        nc.tensor.matmul(
                            ps_o,
                            lhsT=wo_sb[:, ci, ts(co, 128)],
                            rhs=relu_b[:, ci, :],
                            start=(ci == 0),
                            stop=(ci == NCQ - 1),
                        )
                    osb = osbp.tile([128, BL], f32, tag="osb")
                    nc.scalar.add(osb, ps_o, bo_sb[:, co:co + 1])
                    nc.gpsimd.dma_start(outr[:, co, ts(b, BL)], osb)

    nc.finalize()
    return nc


def _part_major(v: np.ndarray) -> np.ndarray:
    """(n*128,) f32 vector -> (128, n) partition-major layout."""
    return np.ascontiguousarray(v.reshape(-1, 128).T).astype(np.float32)


def make_in_maps(x1, mask, wq, bq, wk, bk, wv, bv, wo, bo):
    X = np.asarray(x1[0], dtype=np.float32).astype(BF16)  # (C, L)
    Xp = np.zeros((C, L + 2 * HALF), BF16)
    Xp[:, HALF:HALF + L] = X

    wqT = np.ascontiguousarray(np.asarray(wq, np.float32).T).astype(BF16)
    wkT = np.ascontiguousarray(np.asarray(wk, np.float32).T).astype(BF16)
    wvT = np.ascontiguousarray(np.asarray(wv, np.float32).T).astype(BF16)
    woT = np.ascontiguousarray(np.asarray(wo, np.float32).T).astype(BF16)
    bqd = _part_major(np.asarray(bq, np.float32))
    bkd = _part_major(np.asarray(bk, np.float32))
    bvd = _part_major(np.asarray(bv, np.float32))
    bod = _part_major(np.asarray(bo, np.float32))

    # additive log-mask per global block: log(window_mask * padded_mask + 1e-9)
    pmpad = np.zeros(L + 2 * HALF, np.float32)
    pmpad[HALF:HALF + L] = np.asarray(mask, np.float32)[0, 0]
    wmcol = np.ones(WIN, np.float32)
    wmcol[-1] = 0.0
    nb_glob = L // BL
    fm = np.stack([wmcol * pmpad[bg * BL: bg * BL + WIN]
                   for bg in range(nb_glob)])  # (32, 1024)
    am_all = np.log(fm + 1e-9).astype(np.float32)

    in_maps = []
    for core in range(NCORES):
        x1sl = np.ascontiguousarray(Xp[:, core * LCORE: core * LCORE + LH])
        amc = am_all[core * NBLK:(core + 1) * NBLK]          # (4, 1024)
        amd = amc.reshape(NBLK, WIN // 128, 128).transpose(2, 0, 1)
        amd = np.ascontiguousarray(amd.reshape(128, NBLK * (WIN // 128)))
        in_maps.append({
            "x1s": x1sl, "wqT": wqT, "wkT": wkT, "wvT": wvT, "woT": woT,
            "bq": bqd, "bk": bkd, "bv": bvd, "bo": bod, "amask": amd,
        })
    return in_maps


_CACHED = {}


def kernel(**inputs) -> np.ndarray:
    x1 = np.asarray(inputs["x1"])
    mask = np.asarray(inputs["mask"])
    in_maps = make_in_maps(
        x1, mask,
        inputs["wq"], inputs["bq"], inputs["wk"], inputs["bk"],
        inputs["wv"], inputs["bv"], inputs["wo"], inputs["bo"])

    from concourse.bass_utils import run_bass_kernel_spmd

    if "nc" not in _CACHED:
        _CACHED["nc"] = build_kernel()
    nc = _CACHED["nc"]

    res = run_bass_kernel_spmd(nc, in_maps, core_ids=list(range(NCORES)))
    outs = [np.asarray(res.results[i]["out"]) for i in range(NCORES)]
    full = np.concatenate(outs, axis=1)[None]          # (1, C, L)
    full = full * np.asarray(mask, np.float32)[:, 0:1, :]
    return np.ascontiguousarray(full.astype(np.float32))


if __name__ == "__main__":
    nc = build_kernel()
    print("built ok")
